# revision 2
# baseline (speedup 1.0000x reference)
"""Trainium2 Bass kernel for CrossAttention (b=4, p=8, n=512, dim=512, 8 heads x 64).

Sharding: 32 independent (b, p) slices, 4 per core across 8 NeuronCores
(data parallel, no collectives). Weights replicated. Inputs pre-transposed
per-slice to [dim, n] bf16 on the host.

Device dataflow per slice (SBUF tiles are [partition, free]):
  - qT/kT = Wq/Wk-blocks^T @ xT; v = xkvT-blocks^T @ Wv (PE), PSUM drained
    to bf16 (q/k on DVE, v on Pool). v lands strided [jb, h, 65] with a ones
    column per head at position 64.
  - scores per HEAD PAIR: the pair's kT/qT halves live on partitions 0-63
    and 64-127, so the two heads' score matmuls are row-tiled
    (tile_position (0,0) / (64,0)) and run CONCURRENTLY in the PE array --
    ~2x the score throughput vs serial emission. exp on ACT (scale=1/8,
    no max subtraction; scores ~N(0,1)) -> pt bf16.
  - PV: pv += v_ext_jb^T @ pt_jb; row 0 accumulates l = sum_j p
    (the ones column; dims land at rows 64-127) at zero extra PE cost.
  - normalize: DVE reciprocal_approx_fast of psum row 0 (zero-copy) ->
    Pool partition_broadcast -> DVE mul into outT16 (bf16). (The exact
    nc.vector.reciprocal is ~3.2us per [1,512] row on HW -- 6 cyc/elem on
    one partition -- and was the hidden kernel bottleneck.)
  - final: fin[i, f] = outT-blocks^T @ Wo (PE); DVE adds broadcast bias
    during the PSUM->SBUF drain; DVE-issued DMA out per 128-row block.

PE program order is software-pipelined: next-slice projection chunks and
prev-slice final-projection tiles are interleaved between per-pair
scores/PV rounds so the tensor engine never idles while ACT/DVE/Pool run
exp/normalize in the shadow. The prologue splits the first wq/xq DMAs into
a d0 chunk + remainder and spreads dispatches across engine queues so the
first projection matmul starts ~4us earlier.
"""

from collections import deque
from contextlib import ExitStack

import ml_dtypes
import numpy as np

import concourse.bass as bass
import concourse.tile as tile
from concourse import bacc, mybir
from concourse.bass_utils import run_bass_kernel_spmd

F32 = mybir.dt.float32
BF16 = mybir.dt.bfloat16

HEADS = 8
DH = 64
N = 512
DIM = 512
SCALE = DH**-0.5
S = 4  # (b, p) slices per core
N_CORES = 8


def _build_body(ctx: ExitStack, tc: tile.TileContext, qT, kvT, wq, wk, wv, wo, bo, out,
                reps: int = 1, pack: bool = True):
    nc = tc.nc
    ST = S * reps

    const = ctx.enter_context(tc.tile_pool(name="const", bufs=1))
    xT = ctx.enter_context(tc.tile_pool(name="xT", bufs=4))
    proj = ctx.enter_context(tc.tile_pool(name="proj", bufs=2))
    ptp = ctx.enter_context(tc.tile_pool(name="ptp", bufs=4))
    outTp = ctx.enter_context(tc.tile_pool(name="outTp", bufs=8))
    nrmp = ctx.enter_context(tc.tile_pool(name="nrmp", bufs=3))
    finp = ctx.enter_context(tc.tile_pool(name="finp", bufs=2))
    mm_ps = ctx.enter_context(tc.tile_pool(name="mm_ps", bufs=2, space="PSUM"))
    st_ps = ctx.enter_context(tc.tile_pool(name="st_ps", bufs=4, space="PSUM"))
    pv_ps = ctx.enter_context(tc.tile_pool(name="pv_ps", bufs=2, space="PSUM"))

    # --- weights (bf16 in DRAM): [512, 512] -> [128, 4*512] ---
    w_sb = {
        name: const.tile([128, 4 * 512], BF16, name=f"{name}16")
        for name in ("wq", "wk", "wv", "wo")
    }
    wq16, wk16, wv16, wo16 = (w_sb[k] for k in ("wq", "wk", "wv", "wo"))

    def dma_w(name, dram, eng=None, d0_split=False):
        eng = eng or nc.sync
        if d0_split:
            # first contraction block alone so the first projection matmul
            # can start after 128KB instead of 512KB
            eng.dma_start(w_sb[name][:, 0:512], dram[0:128, :])
            eng.dma_start(
                w_sb[name][:, 512 : 4 * 512],
                dram[128:512, :].rearrange("(t p) e -> p t e", p=128),
            )
        else:
            eng.dma_start(w_sb[name][:], dram.rearrange("(t p) e -> p t e", p=128))

    bo32 = const.tile([1, 512], F32, name="bo32")
    bob = const.tile([128, 512], F32, name="bob")

    # per-slice state (virtual slice index: reps x S, dram tensors mod S)
    x_tiles = [None] * (ST + 1)
    qkv = [None] * (ST + 1)  # (qT16, kT16, v16)
    pt_tiles = {}  # (s, h) -> pt16
    pv_tiles = {}  # (s, h) -> pv psum
    outT = [None] * (ST + 1)

    def dma_xq(s, eng=None, d0_split=False):
        eng = eng or nc.sync
        xq = xT.tile([128, 4 * 512], BF16, name="xqT")
        if d0_split:
            eng.dma_start(xq[:, 0:512], qT[s % S][0:128, :])
            eng.dma_start(
                xq[:, 512 : 4 * 512],
                qT[s % S][128:512, :].rearrange("(t p) n -> p t n", p=128),
            )
        else:
            eng.dma_start(xq[:], qT[s % S].rearrange("(t p) n -> p t n", p=128))
        return xq

    def dma_xkv(s, eng=None):
        eng = eng or nc.sync
        xkv = xT.tile([128, 4 * 512], BF16, name="xkvT")
        eng.dma_start(xkv[:], kvT[s % S].rearrange("(t p) n -> p t n", p=128))
        return xkv

    def dma_in(s):
        if s >= ST:
            return
        x_tiles[s] = (dma_xq(s), dma_xkv(s))

    def proj_chunks(s):
        """Yield 12 callables, each emitting 4 PE matmuls (+1 drain)."""
        if s >= ST:
            return
        qT16 = proj.tile([128, 4 * 512], BF16, name="qT16")
        kT16 = proj.tile([128, 4 * 512], BF16, name="kT16")
        v16 = proj.tile([128, 4 * 1024], BF16, name="v16")
        # Each head's v block is 128 wide: cols 0-63 all-ones, dims at
        # 64-127. The PV matmul then lands the softmax denominator l at
        # psum partition 0 (reciprocal_approx_fast is base_partition-0
        # only) and the head dims at partitions 64-127 (a PSUM read must
        # start 0/64-aligned; base 32 + span 64 is rejected). Rows 1-63 of
        # the pv psum are l duplicates, never read. The memset survives
        # until the tile rotates; only this memset writes cols 0-63.
        ones_view = v16[:].rearrange("p (j h o) -> p j h o", j=4, h=8)[:, :, :, 0:64]
        nc.gpsimd.memset(ones_view, 1.0)
        qkv[s] = (qT16, kT16, v16)
        xq, xkv = x_tiles[s]

        def qk_chunk(w16, xt, dst, t):
            def emit():
                ps = mm_ps.tile([128, 512], F32, name="mm_ps")
                for d in range(4):
                    nc.tensor.matmul(
                        ps[:],
                        w16[:, d * 512 + t * 128 : d * 512 + (t + 1) * 128],
                        xt[:, d * 512 : (d + 1) * 512],
                        start=(d == 0),
                        stop=(d == 3),
                    )
                nc.vector.tensor_copy(dst[:, t * 512 : (t + 1) * 512], ps[:])
            return emit

        def v_chunk(jb):
            def emit():
                ps = mm_ps.tile([128, 512], F32, name="mm_ps")
                for d in range(4):
                    nc.tensor.matmul(
                        ps[:],
                        xkv[:, d * 512 + jb * 128 : d * 512 + (jb + 1) * 128],
                        wv16[:, d * 512 : (d + 1) * 512],
                        start=(d == 0),
                        stop=(d == 3),
                    )
                dst = v16[:, jb * 1024 : (jb + 1) * 1024]
                dst = dst.rearrange("p (h o) -> p h o", h=8)[:, :, 64:128]
                nc.vector.tensor_copy(dst, ps[:].rearrange("p (h o) -> p h o", h=8))
            return emit

        for t in range(4):
            yield qk_chunk(wq16, xq, qT16, t)
            yield qk_chunk(wk16, xkv, kT16, t)
            yield v_chunk(t)

    def scores_one(s, h, jb, pack=True):
        """One score matmul + exp for head h, key-block jb. The head's
        partition half gives the PE row group: even heads rows 0-63, odd
        heads rows 64-127, so adjacent even/odd emissions run concurrently."""
        qT16, kT16, _ = qkv[s]
        tp, half = h // 2, (h % 2) * 64
        kT_h = kT16[half : half + 64, tp * 512 : (tp + 1) * 512]
        qT_h = qT16[half : half + 64, tp * 512 : (tp + 1) * 512]
        pt16 = pt_tiles[(s, h)]
        stt = st_ps.tile([128, 512], F32, name="st_ps")
        kw = dict(tile_position=(half, 0)) if pack else {}
        nc.tensor.matmul(
            stt[:], kT_h[:, jb * 128 : (jb + 1) * 128], qT_h,
            start=True, stop=True, **kw,
        )
        nc.scalar.activation(
            pt16[:, jb * 512 : (jb + 1) * 512],
            stt[:],
            mybir.ActivationFunctionType.Exp,
            scale=SCALE,
        )

    def pv(s, h):
        _, _, v16 = qkv[s]
        pt16 = pt_tiles.pop((s, h))
        pvt = pv_ps.tile([128, 512], F32, name="pv_ps")
        for jb in range(4):
            nc.tensor.matmul(
                pvt[:],
                v16[:, jb * 1024 + h * 128 : jb * 1024 + (h + 1) * 128],
                pt16[:, jb * 512 : (jb + 1) * 512],
                start=(jb == 0),
                stop=(jb == 3),
            )
        pv_tiles[(s, h)] = pvt

    def normalize(s, h):
        """The softmax denominator sits in psum partition 0 (the ones
        column is v-block index 0), where reciprocal_approx_fast reads PSUM
        directly. The exact nc.vector.reciprocal costs ~6 cycles/element on
        one partition -- ~3.2us per [1,512] row on HW -- and was the hidden
        kernel bottleneck; the approx runs ~1 cpe and its ~51-ULP error is
        far below the bf16 noise floor. (The custom uop is base_partition-0
        only: it miscomputes at any other partition offset.)"""
        tp, half = h // 2, (h % 2) * 64
        if outT[s] is None:
            outT[s] = [
                outTp.tile([128, 512], BF16, name=f"outT16_{t}") for t in range(4)
            ]
        pvt = pv_tiles.pop((s, h))
        rinv = nrmp.tile([1, 512], F32, name="rinv")
        nc.vector.reciprocal_approx_fast(rinv[:], pvt[0:1, :])
        rb = nrmp.tile([64, 512], F32, name="rb")
        nc.gpsimd.partition_broadcast(rb[:], rinv[:])
        nc.vector.tensor_mul(
            outT[s][tp][half : half + 64, :],
            pvt[64:128, :],
            rb[:],
        )

    def final_tiles(s):
        """Yield 4 callables, each emitting one fin psum tile (4 mm + drain)."""
        outT16 = outT[s]

        def fin_chunk(ib):
            def emit():
                ps = mm_ps.tile([128, 512], F32, name="mm_ps")
                for t in range(4):
                    nc.tensor.matmul(
                        ps[:],
                        outT16[t][:, ib * 128 : (ib + 1) * 128],
                        wo16[:, t * 512 : (t + 1) * 512],
                        start=(t == 0),
                        stop=(t == 3),
                    )
                fin = finp.tile([128, 512], F32, name="fin")
                nc.vector.tensor_add(fin[:], ps[:], bob[:])
                nc.sync.dma_start(out[s % S][ib * 128 : (ib + 1) * 128, :], fin[:])
            return emit

        for ib in range(4):
            yield fin_chunk(ib)

    # ---- emission ----
    # PE warmup first (no DMA deps): dummy matmuls on memset data while the
    # first DMAs land -- the p-state ramp completes before the first real
    # matmul and PE continuity bridges straight into the prologue.
    warm = const.tile([128, 512], BF16, name="warm")
    nc.gpsimd.memset(warm[:], 1.0)
    for _ in range(8):
        wps = mm_ps.tile([128, 512], F32, name="mm_ps")
        nc.tensor.matmul(wps[:], warm[:, 0:128], warm[:], start=True, stop=True)

    # Startup DMAs: first projection chunk needs wq-d0 + xq-d0 only; those
    # two go first. Dispatches are spread across engine queues (SP, Pool,
    # ACT, DVE are all idle here) so they don't serialize on one sequencer.
    dma_w("wq", wq, eng=nc.sync, d0_split=True)
    xq0 = dma_xq(0, eng=nc.gpsimd, d0_split=True)
    dma_w("wk", wk, eng=nc.scalar)
    xkv0 = dma_xkv(0, eng=nc.scalar)
    dma_w("wv", wv, eng=nc.sync)
    dma_w("wo", wo, eng=nc.gpsimd)
    x_tiles[0] = (xq0, xkv0)

    def split_chunks(s):
        """12 proj chunks -> (early8 emitted a slice ahead, own4 kept for
        slice s's own rounds; t2/t3 q/k blocks are only needed from pair 2
        on)."""
        c = list(proj_chunks(s))
        if not c:
            return [], []
        # arrival-order for the slice-0 prologue: q chunks need only wq+xq
        early8 = [c[0], c[3], c[1], c[4], c[2], c[5], c[8], c[11]]
        own4 = [c[6], c[7], c[9], c[10]]
        return early8, own4

    early8, own4 = split_chunks(0)
    for chunk in early8:  # prologue, not interleaved
        chunk()
    dma_in(1)
    # bias is first needed by fin(0) drains ~30us in; keep its DMA out of
    # the startup critical path
    nc.sync.dma_start(bo32[:], bo.rearrange("(o f) -> o f", o=1))
    nc.gpsimd.partition_broadcast(bob[:], bo32[:])
    own4_next = own4

    for s in range(ST):
        early8, own4_future = split_chunks(s + 1)
        fin4 = list(final_tiles(s - 1)) if s >= 1 else []
        # 16 filler chunks spread over 4 pair-rounds (4 slots each):
        # own4 first (needed for this slice's pairs 2-3 scores), then the
        # next slice's early chunks and the previous slice's fin tiles.
        fillers = deque(own4_next + early8[:2] + fin4 + early8[2:])
        own4_next = own4_future
        for hp in range(4):  # head pairs (2hp, 2hp+1)
            hA, hB = 2 * hp, 2 * hp + 1
            # pt tiles allocated up front; filled jb-by-jb below
            pt_tiles[(s, hA)] = ptp.tile([128, 4 * 512], BF16, name="pt16")
            pt_tiles[(s, hB)] = ptp.tile([128, 4 * 512], BF16, name="pt16")
            # round layout keeps PE fed while ACT drains the paired score
            # psum banks: filler, 2 packed score pairs, filler, 2 more,
            # prev-pair PV between them.
            if pack:
                order = [(hA, 0), (hB, 0), None, (hA, 1), (hB, 1), "pvA",
                         (hA, 2), (hB, 2), None, (hA, 3), (hB, 3), "pvB"]
            else:
                order = [(hA, 0), (hA, 1), None, (hA, 2), (hA, 3), "pvA",
                         (hB, 0), (hB, 1), None, (hB, 2), (hB, 3), "pvB"]
            if fillers:
                fillers.popleft()()
            for item in order:
                if item is None:
                    if fillers:
                        fillers.popleft()()
                elif item == "pvA":
                    if hp >= 1:
                        pv(s, hA - 2)
                    elif fillers:
                        fillers.popleft()()
                elif item == "pvB":
                    if hp >= 1:
                        pv(s, hB - 2)
                        normalize(s, hA - 2)
                        normalize(s, hB - 2)
                    elif fillers:
                        fillers.popleft()()
                else:
                    scores_one(s, item[0], item[1], pack)
            if hp == 2:
                dma_in(s + 2)
        while fillers:
            fillers.popleft()()
        pv(s, 6)
        pv(s, 7)
        normalize(s, 6)
        normalize(s, 7)
    for chunk in final_tiles(ST - 1):
        chunk()


def build_nc(reps: int = 1, pack: bool = True):
    nc = bacc.Bacc("TRN2", target_bir_lowering=False, debug=False)
    qT = nc.dram_tensor("qT", [S, DIM, N], BF16, kind="ExternalInput").ap()
    kvT = nc.dram_tensor("kvT", [S, DIM, N], BF16, kind="ExternalInput").ap()
    wq = nc.dram_tensor("wq", [DIM, DIM], BF16, kind="ExternalInput").ap()
    wk = nc.dram_tensor("wk", [DIM, DIM], BF16, kind="ExternalInput").ap()
    wv = nc.dram_tensor("wv", [DIM, DIM], BF16, kind="ExternalInput").ap()
    wo = nc.dram_tensor("wo", [DIM, DIM], BF16, kind="ExternalInput").ap()
    bo = nc.dram_tensor("bo", [DIM], F32, kind="ExternalInput").ap()
    out = nc.dram_tensor("out", [S, N, DIM], F32, kind="ExternalOutput").ap()
    with tile.TileContext(nc) as tc:
        with ExitStack() as ctx:
            _build_body(ctx, tc, qT, kvT, wq, wk, wv, wo, bo, out, reps=reps, pack=pack)
    nc.compile()
    return nc


_NC = None
BF = ml_dtypes.bfloat16


def make_in_maps(q_in, kv_in, Wq, Wk, Wv, Wo, bo):
    # host-side layout prep: per-slice transpose to [dim, n] + bf16 cast
    q = np.asarray(q_in, dtype=np.float32).reshape(32, N, DIM)
    kv = np.asarray(kv_in, dtype=np.float32).reshape(32, N, DIM)
    qT = np.ascontiguousarray(q.transpose(0, 2, 1)).astype(BF)
    kvT = np.ascontiguousarray(kv.transpose(0, 2, 1)).astype(BF)
    w = {
        "wq": np.asarray(Wq, dtype=np.float32).astype(BF),
        "wk": np.asarray(Wk, dtype=np.float32).astype(BF),
        "wv": np.asarray(Wv, dtype=np.float32).astype(BF),
        "wo": np.asarray(Wo, dtype=np.float32).astype(BF),
        "bo": np.asarray(bo, dtype=np.float32),
    }
    return [
        {"qT": qT[S * c : S * (c + 1)], "kvT": kvT[S * c : S * (c + 1)], **w}
        for c in range(N_CORES)
    ]


def kernel(q_in, kv_in, Wq, Wk, Wv, Wo, bo):
    global _NC
    if _NC is None:
        _NC = build_nc()
    in_maps = make_in_maps(q_in, kv_in, Wq, Wk, Wv, Wo, bo)
    res = run_bass_kernel_spmd(_NC, in_maps, list(range(N_CORES))).results
    out = np.concatenate([res[c]["out"] for c in range(N_CORES)], axis=0)
    return out.reshape(4, 8, N, DIM)
